# revision 27
# baseline (speedup 1.0000x reference)
"""CosineDistanceLoss kernel for Trainium2 (8 NeuronCores, Bass).

reference: mean_n(1 - sum_d feats[d,n] * warped_feats[d,n])
         = 1 - (1/N) * sum_{d,n} feats[d,n] * warped_feats[d,n]

The loss is a single global sum of the elementwise product, so ANY disjoint
partition of elements across cores is valid. The kernel is pure HBM streaming
(64 MiB/stack total; DVE has ~3x slack), and the measured per-core HBM
bandwidth is ASYMMETRIC and partly stable (nc0 sustains ~320 GB/s while its
stack partner nc1 gets ~401 GB/s; other cores land in 335-402). Since the
graded time is the MAX over cores, we balance: each core gets a slice of the
global element stream sized proportionally to its measured bandwidth.

Mechanics (one NEFF for all cores, shapes must match):
  - The global 2^25-element stream per tensor is cut into 256 chunks of
    128x1024 (0.5 MiB). Core i takes n_i consecutive chunks (sum n_i = 256),
    packed by the host into a [128, CAP*1024] DRAM buffer (first n_i*1024
    cols are real data, rest never read).
  - The kernel schedule has 36 units of capacity: 12 always-active "big"
    slots of 2048 cols (8 KB descriptors — measured ~5% more HBM bandwidth
    than 4 KB) carrying 24 units, plus 12 conditional "small" slots of 1024
    cols giving 1-unit balancing granularity. A core with nact active units
    skips the FIRST (36-nact) small slots (cond DMAs with bounds_check=
    skip_entire_dma: no data moved, semaphore still bumped), so the static
    DVE/sem pipeline is unchanged. The STT for a skipped slot reads garbage
    SBUF into an acc column the host ignores — and because skipped slots
    sit near the FRONT of the schedule (order: big0, big1, smalls,
    big2..big11), those stale STTs run during early-stream DVE slack
    instead of serializing after the last real chunk. The last big slot is
    streamed/processed as 4 quarter-pieces so only ~512 cols of DVE work
    trail the final DMA. nact is a per-core uint32 input pulled into a Sync
    register (~1us HBM ldr) between the big0 and big1 DMA issues, well
    before its first use.
  - Per chunk one fused DVE scalar_tensor_tensor (elementwise mult + free-
    axis add-reduce via accum_out; product discarded through a stride-0
    broadcast output) accumulates into acc[:, j]. Host combines the
    8 x [128, n_i] partials in float64.

Raw hand-rolled semaphores (no TileContext): avoids its ~7us preamble +
~10us epilogue. The NRT-injected postamble (all-sem zeroing, ~7us) and the
const-AP preamble (~2us) are fixed costs outside kernel control.
"""

import os

import numpy as np

import concourse.bacc as bacc
import concourse.mybir as mybir
from concourse.bass_utils import run_bass_kernel_spmd

D, N = 512, 65536
NCORES = 8
P = 128                          # SBUF partitions
TOTAL_ELEMS = D * N              # 2^25 per tensor

FU = 1024                        # allocation unit: 1024 cols = 0.5 MiB/tensor
FB = 2 * FU                      # big-slot width (8 KB descriptors)
NBIG = 12                        # big slots (always active; last one quartered)
NSMALL = 12                      # conditional small slots (FU wide, 4 KB desc)
CAP = 2 * NBIG + NSMALL          # capacity in units = 36
MINCH = 2 * NBIG                 # minimum active units = 24
SB = 6                           # big-slot SBUF ring depth (smalls are resident)
UNIT_ELEMS = P * FU              # 131072
GLOBAL_CHUNKS = TOTAL_ELEMS // UNIT_ELEMS            # 256 units globally

# Per-device chunk counts (jax device order; device i -> physical nc:
# 0->4, 1->5, 2->6, 3->7, 4->2, 5->3, 6->0, 7->1; NC pairs (0,1),(2,3),
# (4,5),(6,7) share an HBM stack). Two empirical rules bake into this:
#   1. nc0 stably sustains only ~310-320 GB/s (others ~370-385), so device 6
#      gets the smallest share and pair totals follow measured stack totals.
#   2. Stack-mates given EQUAL (or ±1) chunk counts phase-lock their
#      identical DMA address strides and the stack collapses to ~600 GB/s
#      total (observed 3/4 such runs); an intra-pair asymmetry of >=2 chunks
#      dephases them and keeps the stack at ~700-760 (3/3 runs). The
#      overloaded mate finishes its excess mostly solo at ~400+ GB/s, so the
#      asymmetry costs ~2-3us while the collapse costs ~15us.
_DEFAULT_N = (30, 33, 32, 34, 32, 34, 27, 34)

IMPL = os.environ.get("COSLOSS_IMPL", "bal")

_CACHE = {}


def _chunk_alloc(weights=None):
    """Per-core chunk counts; default is the hand-tuned static allocation."""
    if weights is None:
        env = os.environ.get("COSLOSS_N")
        if env:
            n = [int(x) for x in env.split(",")]
            assert len(n) == NCORES and sum(n) == GLOBAL_CHUNKS, n
            return n
        wenv = os.environ.get("COSLOSS_WEIGHTS")
        if not wenv:
            n = list(_DEFAULT_N)
            assert sum(n) == GLOBAL_CHUNKS and all(
                MINCH < x <= CAP for x in n
            ), n
            return n
        weights = [float(x) for x in wenv.split(",")]
    w = np.asarray(weights, dtype=np.float64)
    exact = GLOBAL_CHUNKS * w / w.sum()
    n = np.floor(exact).astype(int)
    rem = exact - n
    for i in np.argsort(-rem)[: GLOBAL_CHUNKS - n.sum()]:
        n[i] += 1
    n = np.clip(n, MINCH + 1, CAP)
    # rebalance if clipping broke the sum (shift to/from the largest slots)
    while n.sum() != GLOBAL_CHUNKS:
        if n.sum() < GLOBAL_CHUNKS:
            i = np.argmin(n / w)
            assert n[i] < CAP
            n[i] += 1
        else:
            i = np.argmax(n / w)
            assert n[i] > MINCH + 1
            n[i] -= 1
    assert n.sum() == GLOBAL_CHUNKS and (n > MINCH).all() and (n <= CAP).all(), n
    return [int(x) for x in n]


def _schedule():
    """Slot schedule shared by the kernel builder and the host packer.

    Slots in issue order: big0, big1 (unconditional head), 12 conditional
    small slots, big2..big11 (unconditional tail; big11 is processed as 4
    quarter-pieces so only ~512 cols of DVE work trail the final DMA).
    Each *piece* is one (f-DMA, w-DMA, dsem, STT, acc col) tuple.
    """
    bigs = []
    for b in range(NBIG):
        pieces = [(0, FB)] if b < NBIG - 1 else [(i * FB // 4, FB // 4) for i in range(4)]
        bigs.append(
            dict(
                src=b * FB,
                w=FB,
                tile="big",
                ring=(b % SB) * FB,
                big_idx=b,
                cond_u=None,
                pieces=pieces,
            )
        )
    smalls = [
        dict(
            src=NBIG * FB + u * FU,
            w=FU,
            tile="small",
            ring=u * FU,
            big_idx=None,
            cond_u=u,
            pieces=[(0, FU)],
        )
        for u in range(NSMALL)
    ]
    sched = bigs[0:2] + smalls + bigs[2:]
    # annotate cumulative piece indices
    p = 0
    for e in sched:
        e["piece0"] = p
        p += len(e["pieces"])
    return sched, p  # p = total piece count (27)


def _build_balanced():
    import contextlib

    nc = bacc.Bacc(None)
    sched, npieces = _schedule()
    ncols = NBIG * FB + NSMALL * FU
    f_in = nc.declare_dram_parameter("feats", [P, ncols], mybir.dt.float32, isOutput=False)
    w_in = nc.declare_dram_parameter("warped", [P, ncols], mybir.dt.float32, isOutput=False)
    nact_in = nc.declare_dram_parameter("nact", [1, 1], mybir.dt.uint32, isOutput=False)
    out = nc.declare_dram_parameter(
        "partial", [P, npieces], mybir.dt.float32, isOutput=True
    )

    head = npieces - 4  # acc cols written out early vs at the end
    # last-piece index of each big slot, for ring WAR waits
    big_last_piece = {
        e["big_idx"]: e["piece0"] + len(e["pieces"]) - 1
        for e in sched
        if e["tile"] == "big"
    }
    sbuf_bytes = (SB * FB + NSMALL * FU) * 4 * 2
    assert sbuf_bytes <= 200 * 1024, sbuf_bytes

    with (
        nc.sbuf_tensor([P, SB * FB], mybir.dt.float32) as fbig,
        nc.sbuf_tensor([P, SB * FB], mybir.dt.float32) as wbig,
        nc.sbuf_tensor([P, NSMALL * FU], mybir.dt.float32) as fsml,
        nc.sbuf_tensor([P, NSMALL * FU], mybir.dt.float32) as wsml,
        nc.sbuf_tensor([P, npieces], mybir.dt.float32) as acc,
        nc.sbuf_tensor([P, 1], mybir.dt.float32) as dummy,
    ):
        ftiles = {"big": fbig, "small": fsml}
        wtiles = {"big": wbig, "small": wsml}
        with contextlib.ExitStack() as ctx:
            dsems = [
                ctx.enter_context(nc.semaphore(f"dsem{p}")) for p in range(npieces)
            ]
            vsem = ctx.enter_context(nc.semaphore("vsem"))
            osem = ctx.enter_context(nc.semaphore("osem"))
            nact_reg = ctx.enter_context(nc.sync.register("nact_reg"))
            sem_nums = sorted(s.num for s in [*dsems, vsem, osem])
            assert sem_nums == list(
                range(sem_nums[0], sem_nums[0] + len(sem_nums))
            ), sem_nums
            sem_range = range(sem_nums[0], sem_nums[-1] + 1)

            with nc.Block(no_gpsimd_drain=True) as block:

                @block.sync
                def _(sync):
                    nact = None
                    for si, e in enumerate(sched):
                        if si == 1:
                            # Load this core's active-unit count while the
                            # big0 data drains; first used by small0's cond,
                            # long after the ~1us HBM ldr lands.
                            sync.reg_load(nact_reg, nact_in[0:1, 0:1])
                            nact = sync.snap(nact_reg, min_val=0, max_val=CAP)
                        b = e["big_idx"]
                        if b is not None and b >= SB:
                            # WAR: this ring slot is being read by the STTs
                            # of big (b-SB); HWDGE issue is FIFO per ring,
                            # so this wait also orders later DMAs behind it.
                            sync.wait_ge(vsem, big_last_piece[b - SB] + 1)
                        # small slot u is skipped iff u < NSMALL - (nact -
                        # MINCH), i.e. active iff nact > MINCH + NSMALL-1 - u
                        kw = (
                            {}
                            if e["cond_u"] is None
                            else {"cond": nact > MINCH + NSMALL - 1 - e["cond_u"]}
                        )
                        ft, wt = ftiles[e["tile"]], wtiles[e["tile"]]
                        for pi, (poff, psz) in enumerate(e["pieces"]):
                            psem = dsems[e["piece0"] + pi]
                            sync.dma_start(
                                ft[:, e["ring"] + poff : e["ring"] + poff + psz],
                                f_in[:, e["src"] + poff : e["src"] + poff + psz],
                                **kw,
                            ).then_inc(psem, 16)
                            sync.dma_start(
                                wt[:, e["ring"] + poff : e["ring"] + poff + psz],
                                w_in[:, e["src"] + poff : e["src"] + poff + psz],
                                **kw,
                            ).then_inc(psem, 16)
                    # Write out the bulk of acc early (overlaps the tail of
                    # the input stream); only the last columns stay on the
                    # post-stream critical path.
                    sync.wait_ge(vsem, head)
                    sync.dma_start(out[:, :head], acc[:, :head]).then_inc(osem, 16)
                    sync.wait_ge(vsem, npieces)
                    sync.dma_start(out[:, head:], acc[:, head:]).then_inc(osem, 16)

                @block.vector
                def _(vector):
                    # out = (ft * 1.0) * wt (discarded via stride-0
                    # broadcast), accum_out = per-partition sum per piece.
                    # A cond-skipped small slot reads garbage SBUF into an
                    # acc column the host masks out; skipped slots sit near
                    # the front of the schedule so their STTs run during
                    # early-stream DVE slack.
                    for e in sched:
                        ft, wt = ftiles[e["tile"]], wtiles[e["tile"]]
                        for pi, (poff, psz) in enumerate(e["pieces"]):
                            p = e["piece0"] + pi
                            vector.wait_ge(dsems[p], 32)
                            lo = e["ring"] + poff
                            nc.vector.scalar_tensor_tensor(
                                dummy[:, :].broadcast_to((P, psz)),
                                ft[:, lo : lo + psz],
                                1.0,
                                wt[:, lo : lo + psz],
                                op0=mybir.AluOpType.mult,
                                op1=mybir.AluOpType.mult,
                                accum_out=acc[:, p : p + 1],
                            ).then_inc(vsem, 1)

                @block.gpsimd
                def _(gpsimd):
                    # osem at its final value implies both out-DMAs landed,
                    # which implies every earlier sem reached its final
                    # value. Reset them so the NEFF is safe to re-execute.
                    gpsimd.wait_ge(osem, 32)
                    gpsimd.dma_reset(sem_range)
                    gpsimd.sem_clear(sem_range)

    nc.finalize()
    return nc


def _build_raw_even():
    """Previous even-shard builder (64 rows/core, F=2048) kept as fallback."""
    nc = bacc.Bacc(None)
    import contextlib

    F0, M0, SLOTS0 = 2048, 32768, 8
    nch = M0 // F0  # 16
    chunks = []
    for j in range(nch):
        if j == nch - 1:
            q = F0 // 4
            for k in range(4):
                chunks.append((j * F0 + k * q, q))
        else:
            chunks.append((j * F0, F0))
    nchunks = len(chunks)
    head = nchunks - 4
    f_in = nc.declare_dram_parameter("feats", [P, M0], mybir.dt.float32, isOutput=False)
    w_in = nc.declare_dram_parameter("warped", [P, M0], mybir.dt.float32, isOutput=False)
    out = nc.declare_dram_parameter("partial", [P, nchunks], mybir.dt.float32, isOutput=True)

    with (
        nc.sbuf_tensor([P, SLOTS0 * F0], mybir.dt.float32) as ftile,
        nc.sbuf_tensor([P, SLOTS0 * F0], mybir.dt.float32) as wtile,
        nc.sbuf_tensor([P, nchunks], mybir.dt.float32) as acc,
        nc.sbuf_tensor([P, 1], mybir.dt.float32) as dummy,
    ):
        with contextlib.ExitStack() as ctx:
            dsems = [ctx.enter_context(nc.semaphore(f"dsem{j}")) for j in range(nchunks)]
            vsem = ctx.enter_context(nc.semaphore("vsem"))
            osem = ctx.enter_context(nc.semaphore("osem"))
            sem_nums = sorted(s.num for s in [*dsems, vsem, osem])
            assert sem_nums == list(range(sem_nums[0], sem_nums[0] + len(sem_nums)))
            sem_range = range(sem_nums[0], sem_nums[-1] + 1)

            with nc.Block(no_gpsimd_drain=True) as block:

                @block.sync
                def _(sync):
                    for j, (off, sz) in enumerate(chunks):
                        s = j % SLOTS0
                        if j >= SLOTS0:
                            sync.wait_ge(vsem, j - SLOTS0 + 1)
                        sync.dma_start(
                            ftile[:, s * F0 : s * F0 + sz], f_in[:, off : off + sz]
                        ).then_inc(dsems[j], 16)
                        sync.dma_start(
                            wtile[:, s * F0 : s * F0 + sz], w_in[:, off : off + sz]
                        ).then_inc(dsems[j], 16)
                    sync.wait_ge(vsem, head)
                    sync.dma_start(out[:, :head], acc[:, :head]).then_inc(osem, 16)
                    sync.wait_ge(vsem, nchunks)
                    sync.dma_start(out[:, head:], acc[:, head:]).then_inc(osem, 16)

                @block.vector
                def _(vector):
                    for j, (off, sz) in enumerate(chunks):
                        s = j % SLOTS0
                        vector.wait_ge(dsems[j], 32)
                        nc.vector.scalar_tensor_tensor(
                            dummy[:, :].broadcast_to((P, sz)),
                            ftile[:, s * F0 : s * F0 + sz],
                            1.0,
                            wtile[:, s * F0 : s * F0 + sz],
                            op0=mybir.AluOpType.mult,
                            op1=mybir.AluOpType.mult,
                            accum_out=acc[:, j : j + 1],
                        ).then_inc(vsem, 1)

                @block.gpsimd
                def _(gpsimd):
                    gpsimd.wait_ge(osem, 32)
                    gpsimd.dma_reset(sem_range)
                    gpsimd.sem_clear(sem_range)

    nc.finalize()
    return nc


def _get_nc(impl=None):
    impl = impl or IMPL
    if impl not in _CACHE:
        _CACHE[impl] = _build_balanced() if impl == "bal" else _build_raw_even()
    return _CACHE[impl]


def _slot_active(e, n_units):
    if e["cond_u"] is None:
        return True
    return e["cond_u"] >= NSMALL - (n_units - MINCH)


def _active_cols(n_units):
    sched, npieces = _schedule()
    cols = []
    for e in sched:
        if _slot_active(e, n_units):
            cols.extend(range(e["piece0"], e["piece0"] + len(e["pieces"])))
    return cols


def _pack(flat, start_unit, n_units):
    sched, _ = _schedule()
    buf = np.zeros((P, NBIG * FB + NSMALL * FU), dtype=np.float32)
    cur = start_unit * UNIT_ELEMS
    for e in sched:
        if not _slot_active(e, n_units):
            continue
        w = e["w"]
        buf[:, e["src"] : e["src"] + w] = flat[cur : cur + P * w].reshape(P, w)
        cur += P * w
    assert cur == (start_unit + n_units) * UNIT_ELEMS
    return buf


def _run(feats, warped_feats, impl=None, **spmd_kwargs):
    feats = np.ascontiguousarray(np.asarray(feats), dtype=np.float32)
    warped = np.ascontiguousarray(np.asarray(warped_feats), dtype=np.float32)
    assert feats.shape == (D, N) and warped.shape == (D, N)
    impl = impl or IMPL

    if impl == "bal":
        n = _chunk_alloc()
        ff, wf = feats.reshape(-1), warped.reshape(-1)
        starts = np.concatenate([[0], np.cumsum(n)])
        in_maps = [
            {
                "feats": _pack(ff, starts[c], n[c]),
                "warped": _pack(wf, starts[c], n[c]),
                "nact": np.array([[n[c]]], dtype=np.uint32),
            }
            for c in range(NCORES)
        ]
    else:
        n = None
        DSHARD, M0 = D // NCORES, 32768
        in_maps = [
            {
                "feats": feats[c * DSHARD : (c + 1) * DSHARD].reshape(P, M0),
                "warped": warped[c * DSHARD : (c + 1) * DSHARD].reshape(P, M0),
            }
            for c in range(NCORES)
        ]
    res = run_bass_kernel_spmd(
        _get_nc(impl), in_maps, core_ids=list(range(NCORES)), **spmd_kwargs
    )
    res.chunk_alloc = n
    return res


def gather_partials(res):
    """Mask-aware reduction of per-core partials to the scalar loss."""
    n = getattr(res, "chunk_alloc", None)
    total = 0.0
    for c, r in enumerate(res.results):
        p = r["partial"].astype(np.float64)
        if n is not None:
            p = p[:, _active_cols(n[c])]
        total += float(p.sum())
    return np.array(1.0 - total / N, dtype=np.float32)


def kernel(feats, warped_feats):
    return gather_partials(_run(feats, warped_feats))


# revision 28
# speedup vs baseline: 1.1120x; 1.1120x over previous
"""CosineDistanceLoss kernel for Trainium2 (8 NeuronCores, Bass).

reference: mean_n(1 - sum_d feats[d,n] * warped_feats[d,n])
         = 1 - (1/N) * sum_{d,n} feats[d,n] * warped_feats[d,n]

The loss is a single global sum of the elementwise product, so ANY disjoint
partition of elements across cores is valid. The kernel is pure HBM streaming
(64 MiB/stack total; DVE has ~3x slack), and the measured per-core HBM
bandwidth is ASYMMETRIC and partly stable (nc0 sustains ~320 GB/s while its
stack partner nc1 gets ~401 GB/s; other cores land in 335-402). Since the
graded time is the MAX over cores, we balance: each core gets a slice of the
global element stream sized proportionally to its measured bandwidth.

Mechanics (one NEFF for all cores, shapes must match):
  - The global 2^25-element stream per tensor is cut into 256 chunks of
    128x1024 (0.5 MiB). Core i takes n_i consecutive chunks (sum n_i = 256),
    packed by the host into a [128, CAP*1024] DRAM buffer (first n_i*1024
    cols are real data, rest never read).
  - The kernel schedule has 36 units of capacity: 12 always-active "big"
    slots of 2048 cols (8 KB descriptors — measured ~5% more HBM bandwidth
    than 4 KB) carrying 24 units, plus 12 conditional "small" slots of 1024
    cols giving 1-unit balancing granularity. A core with nact active units
    skips the FIRST (36-nact) small slots (cond DMAs with bounds_check=
    skip_entire_dma: no data moved, semaphore still bumped), so the static
    DVE/sem pipeline is unchanged. The STT for a skipped slot reads garbage
    SBUF into an acc column the host ignores — and because skipped slots
    sit near the FRONT of the schedule (order: big0, big1, smalls,
    big2..big11), those stale STTs run during early-stream DVE slack
    instead of serializing after the last real chunk. The last big slot is
    streamed/processed as 4 quarter-pieces so only ~512 cols of DVE work
    trail the final DMA. nact is a per-core uint32 input pulled into a Sync
    register (~1us HBM ldr) between the big0 and big1 DMA issues, well
    before its first use.
  - Per chunk one fused DVE scalar_tensor_tensor (elementwise mult + free-
    axis add-reduce via accum_out; product discarded through a stride-0
    broadcast output) accumulates into acc[:, j]. Host combines the
    8 x [128, n_i] partials in float64.

Raw hand-rolled semaphores (no TileContext): avoids its ~7us preamble +
~10us epilogue. The NRT-injected postamble (all-sem zeroing, ~7us) and the
const-AP preamble (~2us) are fixed costs outside kernel control.
"""

import os

import numpy as np

import concourse.bacc as bacc
import concourse.mybir as mybir
from concourse.bass_utils import run_bass_kernel_spmd

D, N = 512, 65536
NCORES = 8
P = 128                          # SBUF partitions
TOTAL_ELEMS = D * N              # 2^25 per tensor

FU = 1024                        # allocation unit: 1024 cols = 0.5 MiB/tensor
FB = 2 * FU                      # big-slot width (8 KB descriptors)
NBIG = 12                        # big slots (always active; last one quartered)
NSMALL = 12                      # conditional small slots (FU wide, 4 KB desc)
CAP = 2 * NBIG + NSMALL          # capacity in units = 36
MINCH = 2 * NBIG                 # minimum active units = 24
SB = 6                           # big-slot SBUF ring depth (smalls are resident)
UNIT_ELEMS = P * FU              # 131072
GLOBAL_CHUNKS = TOTAL_ELEMS // UNIT_ELEMS            # 256 units globally

# Per-device chunk counts (jax device order; device i -> physical nc:
# 0->4, 1->5, 2->6, 3->7, 4->2, 5->3, 6->0, 7->1; NC pairs (0,1),(2,3),
# (4,5),(6,7) share an HBM stack). Two empirical rules bake into this:
#   1. nc0 stably sustains only ~310-320 GB/s (others ~370-385), so device 6
#      gets the smallest share and pair totals follow measured stack totals.
#   2. Stack-mates given EQUAL (or ±1) chunk counts phase-lock their
#      identical DMA address strides and the stack collapses to ~600 GB/s
#      total (observed 3/4 such runs); an intra-pair asymmetry of >=2 chunks
#      dephases them and keeps the stack at ~700-760 (3/3 runs). The
#      overloaded mate finishes its excess mostly solo at ~400+ GB/s, so the
#      asymmetry costs ~2-3us while the collapse costs ~15us.
# Averaged measured GB/s over recent runs: odd NCs ~381-385 (they win the
# stack arbitration under co-saturation), even NCs ~331-349, nc0 ~313.
_DEFAULT_N = (31, 34, 31, 34, 30, 34, 28, 34)

IMPL = os.environ.get("COSLOSS_IMPL", "bal")

_CACHE = {}


def _chunk_alloc(weights=None):
    """Per-core chunk counts; default is the hand-tuned static allocation."""
    if weights is None:
        env = os.environ.get("COSLOSS_N")
        if env:
            n = [int(x) for x in env.split(",")]
            assert len(n) == NCORES and sum(n) == GLOBAL_CHUNKS, n
            return n
        wenv = os.environ.get("COSLOSS_WEIGHTS")
        if not wenv:
            n = list(_DEFAULT_N)
            assert sum(n) == GLOBAL_CHUNKS and all(
                MINCH < x <= CAP for x in n
            ), n
            return n
        weights = [float(x) for x in wenv.split(",")]
    w = np.asarray(weights, dtype=np.float64)
    exact = GLOBAL_CHUNKS * w / w.sum()
    n = np.floor(exact).astype(int)
    rem = exact - n
    for i in np.argsort(-rem)[: GLOBAL_CHUNKS - n.sum()]:
        n[i] += 1
    n = np.clip(n, MINCH + 1, CAP)
    # rebalance if clipping broke the sum (shift to/from the largest slots)
    while n.sum() != GLOBAL_CHUNKS:
        if n.sum() < GLOBAL_CHUNKS:
            i = np.argmin(n / w)
            assert n[i] < CAP
            n[i] += 1
        else:
            i = np.argmax(n / w)
            assert n[i] > MINCH + 1
            n[i] -= 1
    assert n.sum() == GLOBAL_CHUNKS and (n > MINCH).all() and (n <= CAP).all(), n
    return [int(x) for x in n]


def _schedule():
    """Slot schedule shared by the kernel builder and the host packer.

    Slots in issue order: big0, big1 (unconditional head), 12 conditional
    small slots, big2..big11 (unconditional tail; big11 is processed as 4
    quarter-pieces so only ~512 cols of DVE work trail the final DMA).
    Each *piece* is one (f-DMA, w-DMA, dsem, STT, acc col) tuple.
    """
    bigs = []
    for b in range(NBIG):
        pieces = [(0, FB)] if b < NBIG - 1 else [(i * FB // 4, FB // 4) for i in range(4)]
        bigs.append(
            dict(
                src=b * FB,
                w=FB,
                tile="big",
                ring=(b % SB) * FB,
                big_idx=b,
                cond_u=None,
                pieces=pieces,
            )
        )
    smalls = [
        dict(
            src=NBIG * FB + u * FU,
            w=FU,
            tile="small",
            ring=u * FU,
            big_idx=None,
            cond_u=u,
            pieces=[(0, FU)],
        )
        for u in range(NSMALL)
    ]
    sched = bigs[0:2] + smalls + bigs[2:]
    # annotate cumulative piece indices
    p = 0
    for e in sched:
        e["piece0"] = p
        p += len(e["pieces"])
    return sched, p  # p = total piece count (27)


def _build_balanced():
    import contextlib

    nc = bacc.Bacc(None)
    sched, npieces = _schedule()
    ncols = NBIG * FB + NSMALL * FU
    f_in = nc.declare_dram_parameter("feats", [P, ncols], mybir.dt.float32, isOutput=False)
    w_in = nc.declare_dram_parameter("warped", [P, ncols], mybir.dt.float32, isOutput=False)
    nact_in = nc.declare_dram_parameter("nact", [1, 1], mybir.dt.uint32, isOutput=False)
    out = nc.declare_dram_parameter(
        "partial", [P, npieces], mybir.dt.float32, isOutput=True
    )

    head = npieces - 4  # acc cols written out early vs at the end
    # last-piece index of each big slot, for ring WAR waits
    big_last_piece = {
        e["big_idx"]: e["piece0"] + len(e["pieces"]) - 1
        for e in sched
        if e["tile"] == "big"
    }
    sbuf_bytes = (SB * FB + NSMALL * FU) * 4 * 2
    assert sbuf_bytes <= 200 * 1024, sbuf_bytes

    with (
        nc.sbuf_tensor([P, SB * FB], mybir.dt.float32) as fbig,
        nc.sbuf_tensor([P, SB * FB], mybir.dt.float32) as wbig,
        nc.sbuf_tensor([P, NSMALL * FU], mybir.dt.float32) as fsml,
        nc.sbuf_tensor([P, NSMALL * FU], mybir.dt.float32) as wsml,
        nc.sbuf_tensor([P, npieces], mybir.dt.float32) as acc,
        nc.sbuf_tensor([P, 1], mybir.dt.float32) as dummy,
    ):
        ftiles = {"big": fbig, "small": fsml}
        wtiles = {"big": wbig, "small": wsml}
        with contextlib.ExitStack() as ctx:
            dsems = [
                ctx.enter_context(nc.semaphore(f"dsem{p}")) for p in range(npieces)
            ]
            vsem = ctx.enter_context(nc.semaphore("vsem"))
            osem = ctx.enter_context(nc.semaphore("osem"))
            nact_reg = ctx.enter_context(nc.sync.register("nact_reg"))
            sem_nums = sorted(s.num for s in [*dsems, vsem, osem])
            assert sem_nums == list(
                range(sem_nums[0], sem_nums[0] + len(sem_nums))
            ), sem_nums
            sem_range = range(sem_nums[0], sem_nums[-1] + 1)

            with nc.Block(no_gpsimd_drain=True) as block:

                @block.sync
                def _(sync):
                    nact = None
                    for si, e in enumerate(sched):
                        if si == 1:
                            # Load this core's active-unit count while the
                            # big0 data drains; first used by small0's cond,
                            # long after the ~1us HBM ldr lands.
                            sync.reg_load(nact_reg, nact_in[0:1, 0:1])
                            nact = sync.snap(nact_reg, min_val=0, max_val=CAP)
                        b = e["big_idx"]
                        if b is not None and b >= SB:
                            # WAR: this ring slot is being read by the STTs
                            # of big (b-SB); HWDGE issue is FIFO per ring,
                            # so this wait also orders later DMAs behind it.
                            sync.wait_ge(vsem, big_last_piece[b - SB] + 1)
                        # small slot u is skipped iff u < NSMALL - (nact -
                        # MINCH), i.e. active iff nact > MINCH + NSMALL-1 - u
                        kw = (
                            {}
                            if e["cond_u"] is None
                            else {"cond": nact > MINCH + NSMALL - 1 - e["cond_u"]}
                        )
                        ft, wt = ftiles[e["tile"]], wtiles[e["tile"]]
                        for pi, (poff, psz) in enumerate(e["pieces"]):
                            psem = dsems[e["piece0"] + pi]
                            sync.dma_start(
                                ft[:, e["ring"] + poff : e["ring"] + poff + psz],
                                f_in[:, e["src"] + poff : e["src"] + poff + psz],
                                **kw,
                            ).then_inc(psem, 16)
                            sync.dma_start(
                                wt[:, e["ring"] + poff : e["ring"] + poff + psz],
                                w_in[:, e["src"] + poff : e["src"] + poff + psz],
                                **kw,
                            ).then_inc(psem, 16)
                    # Write out the bulk of acc early (overlaps the tail of
                    # the input stream); only the last columns stay on the
                    # post-stream critical path.
                    sync.wait_ge(vsem, head)
                    sync.dma_start(out[:, :head], acc[:, :head]).then_inc(osem, 16)
                    sync.wait_ge(vsem, npieces)
                    sync.dma_start(out[:, head:], acc[:, head:]).then_inc(osem, 16)

                @block.vector
                def _(vector):
                    # out = (ft * 1.0) * wt (discarded via stride-0
                    # broadcast), accum_out = per-partition sum per piece.
                    # A cond-skipped small slot reads garbage SBUF into an
                    # acc column the host masks out; skipped slots sit near
                    # the front of the schedule so their STTs run during
                    # early-stream DVE slack.
                    for e in sched:
                        ft, wt = ftiles[e["tile"]], wtiles[e["tile"]]
                        for pi, (poff, psz) in enumerate(e["pieces"]):
                            p = e["piece0"] + pi
                            vector.wait_ge(dsems[p], 32)
                            lo = e["ring"] + poff
                            nc.vector.scalar_tensor_tensor(
                                dummy[:, :].broadcast_to((P, psz)),
                                ft[:, lo : lo + psz],
                                1.0,
                                wt[:, lo : lo + psz],
                                op0=mybir.AluOpType.mult,
                                op1=mybir.AluOpType.mult,
                                accum_out=acc[:, p : p + 1],
                            ).then_inc(vsem, 1)

                @block.gpsimd
                def _(gpsimd):
                    # osem at its final value implies both out-DMAs landed,
                    # which implies every earlier sem reached its final
                    # value. Reset them so the NEFF is safe to re-execute.
                    gpsimd.wait_ge(osem, 32)
                    gpsimd.dma_reset(sem_range)
                    gpsimd.sem_clear(sem_range)

    nc.finalize()
    return nc


def _build_raw_even():
    """Previous even-shard builder (64 rows/core, F=2048) kept as fallback."""
    nc = bacc.Bacc(None)
    import contextlib

    F0, M0, SLOTS0 = 2048, 32768, 8
    nch = M0 // F0  # 16
    chunks = []
    for j in range(nch):
        if j == nch - 1:
            q = F0 // 4
            for k in range(4):
                chunks.append((j * F0 + k * q, q))
        else:
            chunks.append((j * F0, F0))
    nchunks = len(chunks)
    head = nchunks - 4
    f_in = nc.declare_dram_parameter("feats", [P, M0], mybir.dt.float32, isOutput=False)
    w_in = nc.declare_dram_parameter("warped", [P, M0], mybir.dt.float32, isOutput=False)
    out = nc.declare_dram_parameter("partial", [P, nchunks], mybir.dt.float32, isOutput=True)

    with (
        nc.sbuf_tensor([P, SLOTS0 * F0], mybir.dt.float32) as ftile,
        nc.sbuf_tensor([P, SLOTS0 * F0], mybir.dt.float32) as wtile,
        nc.sbuf_tensor([P, nchunks], mybir.dt.float32) as acc,
        nc.sbuf_tensor([P, 1], mybir.dt.float32) as dummy,
    ):
        with contextlib.ExitStack() as ctx:
            dsems = [ctx.enter_context(nc.semaphore(f"dsem{j}")) for j in range(nchunks)]
            vsem = ctx.enter_context(nc.semaphore("vsem"))
            osem = ctx.enter_context(nc.semaphore("osem"))
            sem_nums = sorted(s.num for s in [*dsems, vsem, osem])
            assert sem_nums == list(range(sem_nums[0], sem_nums[0] + len(sem_nums)))
            sem_range = range(sem_nums[0], sem_nums[-1] + 1)

            with nc.Block(no_gpsimd_drain=True) as block:

                @block.sync
                def _(sync):
                    for j, (off, sz) in enumerate(chunks):
                        s = j % SLOTS0
                        if j >= SLOTS0:
                            sync.wait_ge(vsem, j - SLOTS0 + 1)
                        sync.dma_start(
                            ftile[:, s * F0 : s * F0 + sz], f_in[:, off : off + sz]
                        ).then_inc(dsems[j], 16)
                        sync.dma_start(
                            wtile[:, s * F0 : s * F0 + sz], w_in[:, off : off + sz]
                        ).then_inc(dsems[j], 16)
                    sync.wait_ge(vsem, head)
                    sync.dma_start(out[:, :head], acc[:, :head]).then_inc(osem, 16)
                    sync.wait_ge(vsem, nchunks)
                    sync.dma_start(out[:, head:], acc[:, head:]).then_inc(osem, 16)

                @block.vector
                def _(vector):
                    for j, (off, sz) in enumerate(chunks):
                        s = j % SLOTS0
                        vector.wait_ge(dsems[j], 32)
                        nc.vector.scalar_tensor_tensor(
                            dummy[:, :].broadcast_to((P, sz)),
                            ftile[:, s * F0 : s * F0 + sz],
                            1.0,
                            wtile[:, s * F0 : s * F0 + sz],
                            op0=mybir.AluOpType.mult,
                            op1=mybir.AluOpType.mult,
                            accum_out=acc[:, j : j + 1],
                        ).then_inc(vsem, 1)

                @block.gpsimd
                def _(gpsimd):
                    gpsimd.wait_ge(osem, 32)
                    gpsimd.dma_reset(sem_range)
                    gpsimd.sem_clear(sem_range)

    nc.finalize()
    return nc


def _get_nc(impl=None):
    impl = impl or IMPL
    if impl not in _CACHE:
        _CACHE[impl] = _build_balanced() if impl == "bal" else _build_raw_even()
    return _CACHE[impl]


def _slot_active(e, n_units):
    if e["cond_u"] is None:
        return True
    return e["cond_u"] >= NSMALL - (n_units - MINCH)


def _active_cols(n_units):
    sched, npieces = _schedule()
    cols = []
    for e in sched:
        if _slot_active(e, n_units):
            cols.extend(range(e["piece0"], e["piece0"] + len(e["pieces"])))
    return cols


def _pack(flat, start_unit, n_units):
    sched, _ = _schedule()
    buf = np.zeros((P, NBIG * FB + NSMALL * FU), dtype=np.float32)
    cur = start_unit * UNIT_ELEMS
    for e in sched:
        if not _slot_active(e, n_units):
            continue
        w = e["w"]
        buf[:, e["src"] : e["src"] + w] = flat[cur : cur + P * w].reshape(P, w)
        cur += P * w
    assert cur == (start_unit + n_units) * UNIT_ELEMS
    return buf


def _run(feats, warped_feats, impl=None, **spmd_kwargs):
    feats = np.ascontiguousarray(np.asarray(feats), dtype=np.float32)
    warped = np.ascontiguousarray(np.asarray(warped_feats), dtype=np.float32)
    assert feats.shape == (D, N) and warped.shape == (D, N)
    impl = impl or IMPL

    if impl == "bal":
        n = _chunk_alloc()
        ff, wf = feats.reshape(-1), warped.reshape(-1)
        starts = np.concatenate([[0], np.cumsum(n)])
        in_maps = [
            {
                "feats": _pack(ff, starts[c], n[c]),
                "warped": _pack(wf, starts[c], n[c]),
                "nact": np.array([[n[c]]], dtype=np.uint32),
            }
            for c in range(NCORES)
        ]
    else:
        n = None
        DSHARD, M0 = D // NCORES, 32768
        in_maps = [
            {
                "feats": feats[c * DSHARD : (c + 1) * DSHARD].reshape(P, M0),
                "warped": warped[c * DSHARD : (c + 1) * DSHARD].reshape(P, M0),
            }
            for c in range(NCORES)
        ]
    res = run_bass_kernel_spmd(
        _get_nc(impl), in_maps, core_ids=list(range(NCORES)), **spmd_kwargs
    )
    res.chunk_alloc = n
    return res


def gather_partials(res):
    """Mask-aware reduction of per-core partials to the scalar loss."""
    n = getattr(res, "chunk_alloc", None)
    total = 0.0
    for c, r in enumerate(res.results):
        p = r["partial"].astype(np.float64)
        if n is not None:
            p = p[:, _active_cols(n[c])]
        total += float(p.sum())
    return np.array(1.0 - total / N, dtype=np.float32)


def kernel(feats, warped_feats):
    return gather_partials(_run(feats, warped_feats))
